# revision 1
# baseline (speedup 1.0000x reference)
"""Trainium2 Bass kernel for nn_ProjectLoss (bce + min-dist affinity loss).

Reference computes, per (b,h,w):
  loss        = -g*ln(p+EPS) - (1-g)*ln(|1-p-EPS|)
  min_dist    = min_{ij} [ gt_th * (grid[h,w,i,j]+1) * p ],   gt_th = g + (1-g)*BIG
  min_dist_inv= min_{ij} [ g * (grid[h,w,i,j]+1) * pm ],      pm    = p + (1-p)*BIG

Since gt_th, p, g, pm >= 0 and fp32 rounding is monotone, the min over (i,j)
factors bit-exactly:
  min_{ij} fl(fl(c0*fl(d_ij+1))*c1) = fl(fl(c0*fl(min_ij d_ij + 1))*c1)
so the whole [B,H,W,64,64] reduction collapses to a row-min of the raw grid
(md_raw[h,w] = min_ij grid[h,w,i,j]) followed by a tiny elementwise epilogue.

Sharding: grid [64,64,64,64] split along H across 8 cores -> per-core
[8,64,64,64] viewed as [512,4096]; preds/gts sliced to the same 8 h-rows and
pre-transposed on host into the kernel's (partition=hw%...) layout.
"""

import sys

sys.path.insert(0, "/opt/trn_rl_repo")

import numpy as np
from contextlib import ExitStack

import concourse.bass as bass
from concourse import mybir
from concourse.bass_utils import run_bass_kernel_spmd

EPS = 1e-08
BIG = 1000000.0
F32 = mybir.dt.float32
AF = mybir.ActivationFunctionType
ALU = mybir.AluOpType
AX = mybir.AxisListType

N_CORES = 8
B, H, W = 2, 64, 64
HC = H // N_CORES          # h-rows per core = 8
ROWS = HC * W              # (h,w) pairs per core = 512
COLS = W * W               # (i,j) per (h,w) = 4096
RB = ROWS // 128           # row blocks of 128 partitions = 4
CS = 2                     # free-dim splits per row block
CW = COLS // CS            # 2048

_NC_CACHE = {}


# Free-dim chunking of the per-core [512, 4096] grid: 1 MiB chunks
# ([128, 2048]).  Measured: DVE reduce 2.29 us/chunk vs DMA 2.93 us/chunk
# (22% slack, no backlog); smaller chunks push per-op reduce overhead above
# the DMA rate and DVE becomes the bottleneck.  The last row block tapers
# (1024, 512, 512) so the post-stream reduce tail is short.
CHUNKS = [
    (0, 0, 2048), (0, 2048, 2048),
    (1, 0, 2048), (1, 2048, 2048),
    (2, 0, 2048), (2, 2048, 2048),
    (3, 0, 2048), (3, 2048, 1024), (3, 3072, 512), (3, 3584, 512),
]
# vseq value after the md4 TS: one inc per chunk reduce, per row-block
# combine, plus the TS itself
MD4_VSEQ = len(CHUNKS) + RB + 1


def _build():
    """Raw Bass program (no Tile): manual engines + semaphores.

    sync   : grid-chunk DMA triggers (HWDGE, FIFO -> chunks stream in order
             at the ~358 GB/s per-core HBM cap) + the two out DMAs
    scalar : pg DMA, ACT epilogue (1-p, 1-g, ln, *BIG)
    gpsimd : eps consts, bce-loss combine, gt_th/pm, one min_dist_inv pair
    vector : per-chunk min reduces, per-rowblock combines, md4 = min+1,
             min_dist products + other min_dist_inv pair
    """
    # Skip the Bass-init all-engine barrier: it only protects the 0.0/1.0
    # const APs, which this program never reads (biases are explicit tiles or
    # float immediates).  Saves ~6 us of GpSimd-boot wait before the first
    # grid DMA trigger.
    _orig_barrier = bass.Bass.all_engine_barrier
    try:
        bass.Bass.all_engine_barrier = lambda self, *a, **k: None
        nc = bass.Bass("TRN2", target_bir_lowering=False, debug=False,
                       num_devices=N_CORES)
    finally:
        bass.Bass.all_engine_barrier = _orig_barrier
    grid = nc.declare_dram_parameter("grid", [ROWS, COLS], F32, isOutput=False)
    pg = nc.declare_dram_parameter("pg", [128, 16], F32, isOutput=False)
    out = nc.declare_dram_parameter("out", [128, 24], F32, isOutput=True)

    gt_tiles = [
        nc.alloc_sbuf_tensor(f"gchunk{k}", [128, w], F32).ap()
        for k, (_, _, w) in enumerate(CHUNKS)
    ]
    sb = lambda name, shape: nc.alloc_sbuf_tensor(name, shape, F32).ap()
    pgt = sb("pgt", [128, 16])
    p = pgt[:, 0:8]
    g = pgt[:, 8:16]
    ot = sb("ot", [128, 24])
    cb = sb("cb", [128, 2])
    lnp = sb("lnp", [128, 8])
    omp = sb("omp", [128, 8])
    ab = sb("ab", [128, 8])
    ln2 = sb("ln2", [128, 8])
    omg = sb("omg", [128, 8])
    u = sb("u", [128, 8])
    v = sb("v", [128, 8])
    s = sb("s", [128, 8])
    c1 = sb("c1", [128, 8])
    d1 = sb("d1", [128, 8])
    gt_th = sb("gt_th", [128, 8])
    pm = sb("pm", [128, 8])
    md4r = sb("md4r", [128, RB])
    part = sb("part", [128, 16])
    md4 = sb("md4", [128, RB])
    tmp = sb("tmp", [128, 8])
    tmp2 = sb("tmp2", [128, 8])
    tmpv = sb("tmpv", [128, 4])

    with ExitStack() as ctx:
        block = ctx.enter_context(nc.Block())
        gsem = [ctx.enter_context(nc.semaphore(f"gsem{k}"))
                for k in range(len(CHUNKS))]
        psem = ctx.enter_context(nc.semaphore("psem"))
        csem = ctx.enter_context(nc.semaphore("csem"))
        asem = ctx.enter_context(nc.semaphore("asem"))
        gseq = ctx.enter_context(nc.semaphore("gseq"))
        vseq = ctx.enter_context(nc.semaphore("vseq"))
        vdone = ctx.enter_context(nc.semaphore("vdone"))
        gdone = ctx.enter_context(nc.semaphore("gdone"))
        osem = ctx.enter_context(nc.semaphore("osem"))

        @block.sync
        def _(sync: bass.BassEngine):
            for k, (i, off, w) in enumerate(CHUNKS):
                sync.dma_start(
                    out=gt_tiles[k],
                    in_=grid[128 * i:128 * (i + 1), off:off + w],
                ).then_inc(gsem[k], 16)
            # out DMA on the sync HWDGE ring (ACT ring showed multi-us
            # completion latency); ring is drained by trigger time.  No
            # wait on osem: the write-receipt is ~4 us and the Block-exit
            # drain + NRT teardown + host output fetch give ample ordering
            # slack before anyone reads HBM.
            # loss columns are ready long before md/mdi -> flush them early
            # so the final DMA is smaller and its receipt fully overlapped.
            sync.wait_ge(gseq, 6)   # gp's ot0 (loss) write
            sync.dma_start(out=out[:, 0:8], in_=ot[:, 0:8]).then_inc(osem, 16)
            sync.wait_ge(vdone, 1)
            sync.wait_ge(gdone, 1)
            sync.dma_start(out=out[:, 8:24], in_=ot[:, 8:24]).then_inc(osem, 16)

        @block.scalar
        def _(act: bass.BassEngine):
            act.dma_start(out=pgt, in_=pg[:]).then_inc(psem, 16)
            act.wait_ge(psem, 16)
            act.wait_ge(csem, 2)
            act.activation(omp, p, AF.Copy, bias=1.0, scale=-1.0).then_inc(asem)
            act.activation(omg, g, AF.Copy, bias=1.0, scale=-1.0).then_inc(asem)
            act.activation(lnp, p, AF.Ln, bias=cb[:, 0:1]).then_inc(asem)
            act.wait_ge(asem, 1)
            act.activation(ab, omp, AF.Abs, bias=cb[:, 1:2]).then_inc(asem)
            act.wait_ge(asem, 4)
            act.activation(ln2, ab, AF.Ln).then_inc(asem)
            act.activation(c1, omg, AF.Copy, scale=BIG).then_inc(asem)
            act.activation(d1, omp, AF.Copy, scale=BIG).then_inc(asem)

        @block.gpsimd
        def _(gp: bass.BassEngine):
            gp.memset(cb[:, 0:1], EPS).then_inc(csem)
            gp.memset(cb[:, 1:2], -EPS).then_inc(csem)
            gp.wait_ge(asem, 7)
            gp.tensor_add(gt_th, g, c1).then_inc(gseq)      # 1
            gp.tensor_add(pm, p, d1).then_inc(gseq)         # 2
            gp.wait_ge(gseq, 2)
            gp.tensor_mul(u, g, lnp).then_inc(gseq)         # 3
            gp.tensor_mul(v, omg, ln2).then_inc(gseq)       # 4
            gp.wait_ge(gseq, 4)
            gp.tensor_add(s, u, v).then_inc(gseq)           # 5
            gp.wait_ge(gseq, 5)
            gp.tensor_scalar_mul(ot[:, 0:8], s, -1.0).then_inc(gseq)  # 6
            gp.wait_ge(vseq, MD4_VSEQ)   # md4 ready
            gp.tensor_mul(tmp2[:, 0:4], g[:, 0:4], md4).then_inc(gseq)  # 7
            gp.wait_ge(gseq, 7)
            gp.tensor_mul(ot[:, 16:20], tmp2[:, 0:4],
                          pm[:, 0:4]).then_inc(gdone, 1)

        @block.vector
        def _(vec: bass.BassEngine):
            vq = 0
            rb_first = {}   # row block -> first chunk index (CHUNKS grouped)
            for k, (i, off, w) in enumerate(CHUNKS):
                rb_first.setdefault(i, k)
                vec.wait_ge(gsem[k], 16)
                vec.tensor_reduce(part[:, k:k + 1], gt_tiles[k], axis=AX.X,
                                  op=ALU.min).then_inc(vseq)
                vq += 1
                if k + 1 == len(CHUNKS) or CHUNKS[k + 1][0] != i:
                    vec.wait_ge(vseq, vq)
                    vec.tensor_reduce(md4r[:, i:i + 1],
                                      part[:, rb_first[i]:k + 1], axis=AX.X,
                                      op=ALU.min).then_inc(vseq)
                    vq += 1
            vec.wait_ge(vseq, vq)
            vec.tensor_scalar_add(md4, md4r, 1.0).then_inc(vseq)
            vq += 1
            vec.wait_ge(gseq, 2)   # gt_th+pm ready (gp incs 1,2)
            # DVE takes min_dist (both batches) + min_dist_inv batch 1;
            # GpSimd (slower per-op) takes only min_dist_inv batch 0.
            vec.tensor_mul(tmp[:, 0:4], gt_th[:, 0:4], md4).then_inc(vseq)
            vec.tensor_mul(tmp[:, 4:8], gt_th[:, 4:8], md4).then_inc(vseq)
            vec.tensor_mul(tmpv, g[:, 4:8], md4).then_inc(vseq)
            vq += 3
            vec.wait_ge(vseq, vq)
            vec.tensor_mul(ot[:, 8:12], tmp[:, 0:4], p[:, 0:4]).then_inc(vseq)
            vec.tensor_mul(ot[:, 12:16], tmp[:, 4:8], p[:, 4:8]).then_inc(vseq)
            vq += 2
            vec.wait_ge(vseq, vq)
            vec.tensor_mul(ot[:, 20:24], tmpv,
                           pm[:, 4:8]).then_inc(vdone, 1)

    return nc


def get_nc():
    if "nc" not in _NC_CACHE:
        _NC_CACHE["nc"] = _build()
    return _NC_CACHE["nc"]


def make_in_maps(preds, gts, grid):
    preds = np.ascontiguousarray(np.asarray(preds, dtype=np.float32))
    gts = np.ascontiguousarray(np.asarray(gts, dtype=np.float32))
    grid = np.ascontiguousarray(np.asarray(grid, dtype=np.float32))
    in_maps = []
    for c in range(N_CORES):
        gslice = np.ascontiguousarray(
            grid[HC * c:HC * (c + 1)].reshape(ROWS, COLS))
        pf = preds[:, HC * c:HC * (c + 1), :].reshape(B, ROWS)
        gf = gts[:, HC * c:HC * (c + 1), :].reshape(B, ROWS)
        pg = np.empty((128, 16), np.float32)
        for b in range(B):
            for t in range(RB):
                pg[:, 4 * b + t] = pf[b, 128 * t:128 * (t + 1)]
                pg[:, 8 + 4 * b + t] = gf[b, 128 * t:128 * (t + 1)]
        in_maps.append({"grid": gslice, "pg": pg})
    return in_maps


def unshard(results):
    loss = np.empty((B, H, W), np.float32)
    md = np.empty((B, H, W), np.float32)
    mdi = np.empty((B, H, W), np.float32)
    for c in range(N_CORES):
        o = results[c]["out"]  # [128, 24]
        for b in range(B):
            for t in range(RB):
                rows = slice(128 * t, 128 * (t + 1))
                loss[b, HC * c:HC * (c + 1)].reshape(ROWS)[rows] = o[:, 4 * b + t]
                md[b, HC * c:HC * (c + 1)].reshape(ROWS)[rows] = o[:, 8 + 4 * b + t]
                mdi[b, HC * c:HC * (c + 1)].reshape(ROWS)[rows] = o[:, 16 + 4 * b + t]
    return loss, md, mdi


def run(preds, gts, grid_dist_tensor, trace=False, **trace_kwargs):
    nc = get_nc()
    in_maps = make_in_maps(preds, gts, grid_dist_tensor)
    res = run_bass_kernel_spmd(nc, in_maps, list(range(N_CORES)), trace=trace,
                               **trace_kwargs)
    return unshard(res.results), res


def kernel(**inputs):
    (loss, md, mdi), _ = run(inputs["preds"], inputs["gts"],
                             inputs["grid_dist_tensor"])
    return loss, md, mdi

